# revision 9
# baseline (speedup 1.0000x reference)
"""Sparse (sliding-window) attention head on 8 TRN2 NeuronCores.

Reference computation (B=2, S=4096, D=512, HD=64, SCALE=128):
    q = x @ wq ; k = x @ wk ; v = x @ wv          [B,S,64]
    scores[b,s,w] = q[b,s] . k[b,s-128+w] / 8     w in [0,256), zero-padded OOB
    out = softmax_w(scores) @ v_window            [B,S,64]

Sharding: 8 shards = (batch b, 1024-seq chunk c). Each shard gets a
zero-padded 128-halo of x on both sides, which reproduces the reference's
zero-padded (not masked) window semantics exactly. All compute is local.

v10 layout (per core):
    Inputs arrive bf16 as 4 fat tensors (>=2.5KB/partition descriptors for
    ~200GB/s/queue): T0=[wqk|wv|x 0:128] + T1=[x 512:896] on the SP queue,
    S0=[x 128:512] + S1=[x 896:1280] on the ACT queue.  Warmup MMs hold the
    PE HAM window busy from ~7.1us until T0 lands (~9.6us) so the clock
    reaches 2.4GHz early.  qk proj per segment (wq|wk packed lhsT), k/q
    evacuated with partition-shifting copies (ACT early, DVE late).  Band
    masks from GPSIMD iota + DVE compares.  All sc->exp->mask chains are
    emitted as early as data allows so the trailing AV accumulations run
    back-to-back with no latency serialization; per-pair norms (batched
    reciprocal + scale) stagger output DMAs, ending with one paired {6,7}
    close + DMA.
"""

import sys
import types

import numpy as np
import ml_dtypes

B, S, D = 2, 4096, 512
HD = 64
SCALE = 128
SS = S // 4          # 1024 positions per shard
HP = SCALE           # halo padding each side
NP = SS + 2 * HP     # 1280 padded positions
NKC = NP // 128      # 10 key chunks
NQB = SS // 128      # 8 query blocks
NDC = D // 128       # 4 d-chunks

# x col segments (start, width); seg0 rides inside T0 after the weights
XSEGS = [(0, 256), (256, 384), (640, 384), (1024, 256)]
N_WARMUP = 13

_CACHE = {}


def _ensure_hooks():
    """Register the axon NTFF profile hook; keep artifacts local."""
    if "antenv.axon_hooks" not in sys.modules:
        try:
            from trn_agent_boot.trn_boot import _ntff_profile_via_ctypes

            m = types.ModuleType("antenv.axon_hooks")
            m.get_axon_ntff_profile_hook = lambda: _ntff_profile_via_ctypes(
                "/opt/axon/libaxon_pjrt.so"
            )
            sys.modules["antenv.axon_hooks"] = m
        except Exception:
            pass
    import concourse.bass_utils as bass_utils

    bass_utils.upload_artifacts = lambda tmpdir: tmpdir


def _build_nc():
    import concourse.mybir as mybir
    import concourse.tile as tile
    from concourse import bacc

    bf = mybir.dt.bfloat16
    f32 = mybir.dt.float32
    AF = mybir.ActivationFunctionType

    nc = bacc.Bacc("TRN2", target_bir_lowering=False, debug=False, num_devices=8)

    t0_d = nc.dram_tensor("t0", [128, NDC, 448], bf, kind="ExternalInput")
    s0_d = nc.dram_tensor("s0", [128, NDC, 384], bf, kind="ExternalInput")
    t1_d = nc.dram_tensor("t1", [128, NDC, 384], bf, kind="ExternalInput")
    s1_d = nc.dram_tensor("s1", [128, NDC, 256], bf, kind="ExternalInput")
    out_d = nc.dram_tensor("outp", [128, NQB, HD + 1], f32, kind="ExternalOutput")

    with tile.TileContext(nc) as tc:
        with (
            tc.tile_pool(name="consts", bufs=1) as consts,
            tc.tile_pool(name="xtp", bufs=1) as xtp,
            tc.tile_pool(name="qkp", bufs=1) as qkp,
            tc.tile_pool(name="vgp", bufs=1) as vgp,
            tc.tile_pool(name="exp_p", bufs=6) as exp_p,
            tc.tile_pool(name="emp", bufs=5) as emp,
            tc.tile_pool(name="fin", bufs=4) as fin,
            tc.tile_pool(name="qkps", bufs=2, space="PSUM") as qkps,
            tc.tile_pool(name="vps", bufs=1, space="PSUM") as vps,
            tc.tile_pool(name="scps", bufs=3, space="PSUM") as scps,
            tc.tile_pool(name="avps", bufs=2, space="PSUM") as avps,
        ):
            # ---- DMAs first (fat descriptors, consumption order) ----
            t0 = xtp.tile([128, NDC, 448], bf, tag="t0")
            s0 = xtp.tile([128, NDC, 384], bf, tag="s0")
            t1 = xtp.tile([128, NDC, 384], bf, tag="t1")
            s1 = xtp.tile([128, NDC, 256], bf, tag="s1")

            nc.sync.dma_start(out=t0, in_=t0_d[:, :, :])
            nc.scalar.dma_start(out=s0, in_=s0_d[:, :, :])
            nc.sync.dma_start(out=t1, in_=t1_d[:, :, :])
            nc.scalar.dma_start(out=s1, in_=s1_d[:, :, :])

            def wqk(dc):
                return t0[:, dc, 0:128]

            def wv_(dc):
                return t0[:, dc, 128:192]

            # x segment view: seg si, dc, col slice (cols local to seg)
            segt = [
                (t0, 192),   # seg0: x cols 0:128 at offset 192
                (s0, 0),
                (t1, 0),
                (s1, 0),
            ]

            def xs(si, dc, a, b):
                t, off = segt[si]
                return t[:, dc, off + a : off + b]

            # ---- consts: warmup garbage (DVE), exp-table trigger (ACT),
            # band mask via GPSIMD iota + DVE compares, vaug ones (GPSIMD) ----
            garb = consts.tile([128, 260], bf, tag="garb")
            nc.vector.memset(garb, 0.5)
            zz = consts.tile([128, 1], f32, tag="zz")
            nc.vector.memset(zz, 0.0)
            ez = consts.tile([128, 1], f32, tag="ez")
            nc.scalar.activation(ez, zz, AF.Exp)

            vaug = vgp.tile([128, NKC, 66], bf, tag="vaug")
            nc.gpsimd.memset(vaug[:, :, 64:66], 1.0)
            mi = consts.tile([128, 128], mybir.dt.int16, tag="mi")
            nc.gpsimd.iota(mi, pattern=[[1, 128]], base=0, channel_multiplier=-1)
            mask_s = consts.tile([128, 2, 128], bf, tag="mask")
            nc.vector.tensor_scalar(
                mask_s[:, 0, :], mi, 0, None, mybir.AluOpType.is_le
            )
            nc.vector.tensor_scalar(
                mask_s[:, 1, :], mi, 0, None, mybir.AluOpType.is_gt
            )

            qT_s = qkp.tile([64, SS], bf, tag="qT")
            kT_s = qkp.tile([64, NP], bf, tag="kT")
            ot = fin.tile([128, NQB, HD + 1], f32, tag="ot")

            # ---- PE warmup: hold HAM busy until real data lands ----
            for i in range(N_WARMUP):
                wp = qkps.tile([128, 384], f32, tag="qkps")
                nc.tensor.matmul(
                    wp[:, 0:256],
                    lhsT=garb[:, 0:128],
                    rhs=garb[:, 0:256],
                    start=True,
                    stop=True,
                )

            # ---- helpers ----
            def qk_seg(si):
                a, w = XSEGS[si]
                ps = qkps.tile([128, 384], f32, tag="qkps")
                for dc in range(NDC):
                    nc.tensor.matmul(
                        ps[:, :w],
                        lhsT=wqk(dc),
                        rhs=xs(si, dc, 0, w),
                        start=(dc == 0),
                        stop=(dc == NDC - 1),
                    )
                return ps, a, w

            def evac_seg(ps, a, w, keng, qeng):
                kcp = keng.copy if keng is nc.scalar else keng.tensor_copy
                qcp = qeng.copy if qeng is nc.scalar else qeng.tensor_copy
                kcp(kT_s[:, a : a + w], ps[64:128, :w])
                qa, qb_ = max(a, HP), min(a + w, HP + SS)
                if qa < qb_:
                    qcp(qT_s[:, qa - HP : qb_ - HP], ps[0:64, qa - a : qb_ - a])

            def v_chunk(kc, vp, j):
                c0 = kc * 128
                for si, (a, w) in enumerate(XSEGS):
                    if a <= c0 < a + w:
                        break
                off = c0 - a
                for dc in range(NDC):
                    nc.tensor.matmul(
                        vp[:, j, :],
                        lhsT=xs(si, dc, off, off + 128),
                        rhs=wv_(dc),
                        start=(dc == 0),
                        stop=(dc == NDC - 1),
                    )

            def v_pair(c):
                vp = vps.tile([128, 2, HD], f32, tag="vp")
                v_chunk(c, vp, 0)
                v_chunk(c + 1, vp, 1)
                nc.vector.tensor_copy(vaug[:, c : c + 2, 0:64], vp)

            def sem_block(qb, meng):
                """sc -> exp -> masked-exp for one query block."""
                sc = scps.tile([128, 384], f32, tag="sc")
                for c in range(3):
                    nc.tensor.matmul(
                        sc[:, c * 128 : (c + 1) * 128],
                        lhsT=kT_s[:, (qb + c) * 128 : (qb + c + 1) * 128],
                        rhs=qT_s[:, qb * 128 : (qb + 1) * 128],
                        start=True,
                        stop=True,
                    )
                ex = exp_p.tile([128, 3, 128], bf, tag="ex")
                nc.scalar.activation(ex[:, :, :], sc, AF.Exp, scale=0.125)
                em = emp.tile([128, 2, 128], bf, tag="em")
                meng.tensor_mul(em, ex[:, 0:3:2, :], mask_s)
                return ex, em

            def av_block(qb, exem, av4, j):
                ex, em = exem
                nc.tensor.matmul(
                    av4[:, j, :],
                    lhsT=ex[:, 1, :],
                    rhs=vaug[:, qb + 1, 0:65],
                    start=True,
                    stop=False,
                )
                nc.tensor.matmul(
                    av4[:, j, :],
                    lhsT=em[:, 0, :],
                    rhs=vaug[:, qb, 0:65],
                    start=False,
                    stop=False,
                )
                nc.tensor.matmul(
                    av4[:, j, :],
                    lhsT=em[:, 1, :],
                    rhs=vaug[:, qb + 2, 0:65],
                    start=False,
                    stop=True,
                )

            def close_pair(b, av4, j, eng):
                """Evacuate numerator+denominator; the host does the divide."""
                cp = eng.copy if eng is nc.scalar else eng.tensor_copy
                cp(ot[:, b : b + 2, :], av4[:, j : j + 2, :])

            def filler():
                wp = qkps.tile([128, 384], f32, tag="qkps")
                nc.tensor.matmul(
                    wp[:, 0:256],
                    lhsT=garb[:, 0:128],
                    rhs=garb[:, 0:256],
                    start=True,
                    stop=True,
                )

            # ---- pipeline ----
            EA, EV, EG = nc.scalar, nc.vector, nc.gpsimd

            ps0, a0, w0 = qk_seg(0)
            evac_seg(ps0, a0, w0, EA, EA)
            v_pair(0)
            filler()
            filler()
            ps1, a1, w1 = qk_seg(1)
            evac_seg(ps1, a1, w1, EV, EV)

            ee0 = sem_block(0, EV)
            ps2, a2, w2 = qk_seg(2)
            evac_seg(ps2, a2, w2, EA, EA)
            v_pair(2)

            ee1 = sem_block(1, EG)
            v_pair(4)
            av4a = avps.tile([128, 4, 65], f32, tag="av4")
            av_block(0, ee0, av4a, 0)

            ee2 = sem_block(2, EG)
            av_block(1, ee1, av4a, 1)
            close_pair(0, av4a, 0, EV)
            nc.sync.dma_start(out=out_d[:, 0:2, :], in_=ot[:, 0:2, :])

            ee3 = sem_block(3, EG)
            av_block(2, ee2, av4a, 2)

            ee4 = sem_block(4, EG)
            av_block(3, ee3, av4a, 3)
            close_pair(2, av4a, 2, EA)
            nc.sync.dma_start(out=out_d[:, 2:4, :], in_=ot[:, 2:4, :])

            ps3, a3, w3 = qk_seg(3)
            evac_seg(ps3, a3, w3, EV, EV)
            v_pair(6)
            v_pair(8)

            ee5 = sem_block(5, EV)
            av4b = avps.tile([128, 4, 65], f32, tag="av4")
            av_block(4, ee4, av4b, 0)

            ee6 = sem_block(6, EV)
            av_block(5, ee5, av4b, 1)
            close_pair(4, av4b, 0, EV)
            nc.sync.dma_start(out=out_d[:, 4:6, :], in_=ot[:, 4:6, :])

            ee7 = sem_block(7, EV)
            av_block(6, ee6, av4b, 2)
            av_block(7, ee7, av4b, 3)
            close_pair(6, av4b, 2, EV)
            nc.sync.dma_start(out=out_d[:, 6:8, :], in_=ot[:, 6:8, :])

    nc.compile()
    return nc


def _get_nc():
    if "nc" not in _CACHE:
        _ensure_hooks()
        _CACHE["nc"] = _build_nc()
    return _CACHE["nc"]


def _host_inputs(inputs, wq, wk, wv):
    bf16 = ml_dtypes.bfloat16
    x = np.asarray(inputs, dtype=np.float32)

    # wqkv[p, dc, 0:64]=wq, [64:128]=wk, [128:192]=wv  (rows dc*128+p)
    wcat = np.concatenate(
        [np.asarray(wq), np.asarray(wk), np.asarray(wv)], axis=1
    ).astype(np.float32)                                     # [512, 192]
    wqkv = np.ascontiguousarray(
        wcat.reshape(NDC, 128, 192).transpose(1, 0, 2)
    )                                                        # [128, 4, 192]

    in_maps = []
    for i in range(8):
        b, c = divmod(i, 4)
        s0 = c * SS
        xp = np.zeros((NP, D), np.float32)
        lo = max(0, s0 - HP)
        hi = min(S, s0 + SS + HP)
        xp[lo - (s0 - HP) : hi - (s0 - HP)] = x[b, lo:hi]
        x4 = xp.T.reshape(NDC, 128, NP).transpose(1, 0, 2)  # [128, 4, 1280]
        t0 = np.concatenate([wqkv, x4[:, :, 0:256]], axis=2)  # [128,4,448]
        m = {
            "t0": np.ascontiguousarray(t0).astype(bf16),
            "s0": np.ascontiguousarray(x4[:, :, 256:640]).astype(bf16),
            "t1": np.ascontiguousarray(x4[:, :, 640:1024]).astype(bf16),
            "s1": np.ascontiguousarray(x4[:, :, 1024:1280]).astype(bf16),
        }
        in_maps.append(m)
    return in_maps


def run_sharded(inputs, wq, wk, wv, trace=False, trace_cores=None):
    """Run the SPMD kernel; returns (out [B,S,HD] f32, BassKernelResults)."""
    _ensure_hooks()
    import concourse.bass_utils as bass_utils

    nc = _get_nc()
    in_maps = _host_inputs(inputs, wq, wk, wv)
    res = bass_utils.run_bass_kernel_spmd(
        nc,
        in_maps,
        core_ids=list(range(8)),
        trace=trace,
        trace_cores=trace_cores,
    )
    out = np.empty((B, S, HD), np.float32)
    for i in range(8):
        b, c = divmod(i, 4)
        o = res.results[i]["outp"]                           # [128, 8, 65]
        o = o[:, :, 0:HD] / o[:, :, HD : HD + 1]
        out[b, c * SS : (c + 1) * SS] = o.transpose(1, 0, 2).reshape(SS, HD)
    return out, res


def kernel(inputs, wq, wk, wv):
    out, _ = run_sharded(inputs, wq, wk, wv, trace=False)
    return out


# revision 10
# speedup vs baseline: 1.0324x; 1.0324x over previous
"""Sparse (sliding-window) attention head on 8 TRN2 NeuronCores.

Reference computation (B=2, S=4096, D=512, HD=64, SCALE=128):
    q = x @ wq ; k = x @ wk ; v = x @ wv          [B,S,64]
    scores[b,s,w] = q[b,s] . k[b,s-128+w] / 8     w in [0,256), zero-padded OOB
    out = softmax_w(scores) @ v_window            [B,S,64]

Sharding: 8 shards = (batch b, 1024-seq chunk c). Each shard gets a
zero-padded 128-halo of x on both sides, which reproduces the reference's
zero-padded (not masked) window semantics exactly. All compute is local.

v10 layout (per core):
    Inputs arrive bf16 as 4 fat tensors (>=2.5KB/partition descriptors for
    ~200GB/s/queue): T0=[wqk|wv|x 0:128] + T1=[x 512:896] on the SP queue,
    S0=[x 128:512] + S1=[x 896:1280] on the ACT queue.  Warmup MMs hold the
    PE HAM window busy from ~7.1us until T0 lands (~9.6us) so the clock
    reaches 2.4GHz early.  qk proj per segment (wq|wk packed lhsT), k/q
    evacuated with partition-shifting copies (ACT early, DVE late).  Band
    masks from GPSIMD iota + DVE compares.  All sc->exp->mask chains are
    emitted as early as data allows so the trailing AV accumulations run
    back-to-back with no latency serialization; per-pair norms (batched
    reciprocal + scale) stagger output DMAs, ending with one paired {6,7}
    close + DMA.
"""

import sys
import types

import numpy as np
import ml_dtypes

B, S, D = 2, 4096, 512
HD = 64
SCALE = 128
SS = S // 4          # 1024 positions per shard
HP = SCALE           # halo padding each side
NP = SS + 2 * HP     # 1280 padded positions
NKC = NP // 128      # 10 key chunks
NQB = SS // 128      # 8 query blocks
NDC = D // 128       # 4 d-chunks

# x col segments (start, width); seg0 rides inside T0 after the weights
XSEGS = [(0, 128), (128, 384), (512, 384), (896, 384)]
N_WARMUP = 12

_CACHE = {}


def _ensure_hooks():
    """Register the axon NTFF profile hook; keep artifacts local."""
    if "antenv.axon_hooks" not in sys.modules:
        try:
            from trn_agent_boot.trn_boot import _ntff_profile_via_ctypes

            m = types.ModuleType("antenv.axon_hooks")
            m.get_axon_ntff_profile_hook = lambda: _ntff_profile_via_ctypes(
                "/opt/axon/libaxon_pjrt.so"
            )
            sys.modules["antenv.axon_hooks"] = m
        except Exception:
            pass
    import concourse.bass_utils as bass_utils

    bass_utils.upload_artifacts = lambda tmpdir: tmpdir


def _build_nc():
    import concourse.mybir as mybir
    import concourse.tile as tile
    from concourse import bacc

    bf = mybir.dt.bfloat16
    f32 = mybir.dt.float32
    AF = mybir.ActivationFunctionType

    nc = bacc.Bacc("TRN2", target_bir_lowering=False, debug=False, num_devices=8)

    t0_d = nc.dram_tensor("t0", [128, NDC, 320], bf, kind="ExternalInput")
    s0_d = nc.dram_tensor("s0", [128, NDC, 384], bf, kind="ExternalInput")
    t1_d = nc.dram_tensor("t1", [128, NDC, 384], bf, kind="ExternalInput")
    s1_d = nc.dram_tensor("s1", [128, NDC, 384], bf, kind="ExternalInput")
    out_d = nc.dram_tensor("outp", [128, NQB, HD + 1], f32, kind="ExternalOutput")

    with tile.TileContext(nc) as tc:
        with (
            tc.tile_pool(name="consts", bufs=1) as consts,
            tc.tile_pool(name="xtp", bufs=1) as xtp,
            tc.tile_pool(name="qkp", bufs=1) as qkp,
            tc.tile_pool(name="vgp", bufs=1) as vgp,
            tc.tile_pool(name="exp_p", bufs=6) as exp_p,
            tc.tile_pool(name="emp", bufs=5) as emp,
            tc.tile_pool(name="fin", bufs=4) as fin,
            tc.tile_pool(name="qkps", bufs=2, space="PSUM") as qkps,
            tc.tile_pool(name="vps", bufs=1, space="PSUM") as vps,
            tc.tile_pool(name="scps", bufs=3, space="PSUM") as scps,
            tc.tile_pool(name="avps", bufs=2, space="PSUM") as avps,
        ):
            # ---- DMAs first (fat descriptors, consumption order) ----
            t0 = xtp.tile([128, NDC, 320], bf, tag="t0")
            s0 = xtp.tile([128, NDC, 384], bf, tag="s0")
            t1 = xtp.tile([128, NDC, 384], bf, tag="t1")
            s1 = xtp.tile([128, NDC, 384], bf, tag="s1")

            nc.sync.dma_start(out=t0, in_=t0_d[:, :, :])
            nc.scalar.dma_start(out=s0, in_=s0_d[:, :, :])
            nc.sync.dma_start(out=t1, in_=t1_d[:, :, :])
            nc.scalar.dma_start(out=s1, in_=s1_d[:, :, :])

            def wqk(dc):
                return t0[:, dc, 0:128]

            def wv_(dc):
                return t0[:, dc, 128:192]

            # x segment view: seg si, dc, col slice (cols local to seg)
            segt = [
                (t0, 192),   # seg0: x cols 0:128 at offset 192
                (s0, 0),
                (t1, 0),
                (s1, 0),
            ]

            def xs(si, dc, a, b):
                t, off = segt[si]
                return t[:, dc, off + a : off + b]

            # ---- consts: warmup garbage (DVE), exp-table trigger (ACT),
            # band mask via GPSIMD iota + DVE compares, vaug ones (GPSIMD) ----
            garb = consts.tile([128, 260], bf, tag="garb")
            nc.vector.memset(garb, 0.5)
            zz = consts.tile([128, 1], f32, tag="zz")
            nc.vector.memset(zz, 0.0)
            ez = consts.tile([128, 1], f32, tag="ez")
            nc.scalar.activation(ez, zz, AF.Exp)

            vaug = vgp.tile([128, NKC, 66], bf, tag="vaug")
            nc.gpsimd.memset(vaug[:, :, 64:66], 1.0)
            mi = consts.tile([128, 128], mybir.dt.int16, tag="mi")
            nc.gpsimd.iota(mi, pattern=[[1, 128]], base=0, channel_multiplier=-1)
            mask_s = consts.tile([128, 2, 128], bf, tag="mask")
            nc.vector.tensor_scalar(
                mask_s[:, 0, :], mi, 0, None, mybir.AluOpType.is_le
            )
            nc.vector.tensor_scalar(
                mask_s[:, 1, :], mi, 0, None, mybir.AluOpType.is_gt
            )

            qT_s = qkp.tile([64, SS], bf, tag="qT")
            kT_s = qkp.tile([64, NP], bf, tag="kT")
            ot = fin.tile([128, NQB, HD + 1], f32, tag="ot")

            # ---- PE warmup: hold HAM busy until real data lands ----
            for i in range(N_WARMUP):
                wp = qkps.tile([128, 384], f32, tag="qkps")
                nc.tensor.matmul(
                    wp[:, 0:256],
                    lhsT=garb[:, 0:128],
                    rhs=garb[:, 0:256],
                    start=True,
                    stop=True,
                )

            # ---- helpers ----
            def qk_seg(si):
                a, w = XSEGS[si]
                ps = qkps.tile([128, 384], f32, tag="qkps")
                for dc in range(NDC):
                    nc.tensor.matmul(
                        ps[:, :w],
                        lhsT=wqk(dc),
                        rhs=xs(si, dc, 0, w),
                        start=(dc == 0),
                        stop=(dc == NDC - 1),
                    )
                return ps, a, w

            def evac_seg(ps, a, w, keng, qeng):
                kcp = keng.copy if keng is nc.scalar else keng.tensor_copy
                qcp = qeng.copy if qeng is nc.scalar else qeng.tensor_copy
                kcp(kT_s[:, a : a + w], ps[64:128, :w])
                qa, qb_ = max(a, HP), min(a + w, HP + SS)
                if qa < qb_:
                    qcp(qT_s[:, qa - HP : qb_ - HP], ps[0:64, qa - a : qb_ - a])

            def v_chunk(kc, vp, j):
                c0 = kc * 128
                for si, (a, w) in enumerate(XSEGS):
                    if a <= c0 < a + w:
                        break
                off = c0 - a
                for dc in range(NDC):
                    nc.tensor.matmul(
                        vp[:, j, :],
                        lhsT=xs(si, dc, off, off + 128),
                        rhs=wv_(dc),
                        start=(dc == 0),
                        stop=(dc == NDC - 1),
                    )

            def v_pair(c):
                vp = vps.tile([128, 2, HD], f32, tag="vp")
                v_chunk(c, vp, 0)
                v_chunk(c + 1, vp, 1)
                nc.vector.tensor_copy(vaug[:, c : c + 2, 0:64], vp)

            def sem_block(qb, meng):
                """sc -> exp -> masked-exp for one query block."""
                sc = scps.tile([128, 384], f32, tag="sc")
                for c in range(3):
                    nc.tensor.matmul(
                        sc[:, c * 128 : (c + 1) * 128],
                        lhsT=kT_s[:, (qb + c) * 128 : (qb + c + 1) * 128],
                        rhs=qT_s[:, qb * 128 : (qb + 1) * 128],
                        start=True,
                        stop=True,
                    )
                ex = exp_p.tile([128, 3, 128], bf, tag="ex")
                nc.scalar.activation(ex[:, :, :], sc, AF.Exp, scale=0.125)
                em = emp.tile([128, 2, 128], bf, tag="em")
                meng.tensor_mul(em, ex[:, 0:3:2, :], mask_s)
                return ex, em

            def av_block(qb, exem, av4, j):
                ex, em = exem
                nc.tensor.matmul(
                    av4[:, j, :],
                    lhsT=ex[:, 1, :],
                    rhs=vaug[:, qb + 1, 0:65],
                    start=True,
                    stop=False,
                )
                nc.tensor.matmul(
                    av4[:, j, :],
                    lhsT=em[:, 0, :],
                    rhs=vaug[:, qb, 0:65],
                    start=False,
                    stop=False,
                )
                nc.tensor.matmul(
                    av4[:, j, :],
                    lhsT=em[:, 1, :],
                    rhs=vaug[:, qb + 2, 0:65],
                    start=False,
                    stop=True,
                )

            def close_pair(b, av4, j, eng):
                """Evacuate numerator+denominator; the host does the divide."""
                cp = eng.copy if eng is nc.scalar else eng.tensor_copy
                cp(ot[:, b : b + 2, :], av4[:, j : j + 2, :])

            def filler():
                wp = qkps.tile([128, 384], f32, tag="qkps")
                nc.tensor.matmul(
                    wp[:, 0:256],
                    lhsT=garb[:, 0:128],
                    rhs=garb[:, 0:256],
                    start=True,
                    stop=True,
                )

            # ---- pipeline ----
            EA, EV, EG = nc.scalar, nc.vector, nc.gpsimd

            ps0, a0, w0 = qk_seg(0)
            evac_seg(ps0, a0, w0, EA, EA)
            vp01 = vps.tile([128, 2, HD], f32, tag="vp")
            v_chunk(0, vp01, 0)
            filler()
            filler()
            filler()
            v_chunk(1, vp01, 1)
            nc.vector.tensor_copy(vaug[:, 0:2, 0:64], vp01)

            ps1, a1, w1 = qk_seg(1)
            evac_seg(ps1, a1, w1, EV, EV)
            v_pair(2)

            ee0 = sem_block(0, EV)
            ps2, a2, w2 = qk_seg(2)
            evac_seg(ps2, a2, w2, EA, EA)
            v_pair(4)

            ee1 = sem_block(1, EG)
            av4a = avps.tile([128, 4, 65], f32, tag="av4")
            av_block(0, ee0, av4a, 0)

            ee2 = sem_block(2, EG)
            av_block(1, ee1, av4a, 1)
            close_pair(0, av4a, 0, EV)
            nc.sync.dma_start(out=out_d[:, 0:2, :], in_=ot[:, 0:2, :])

            ee3 = sem_block(3, EG)
            av_block(2, ee2, av4a, 2)

            ee4 = sem_block(4, EG)
            av_block(3, ee3, av4a, 3)
            close_pair(2, av4a, 2, EA)
            nc.scalar.dma_start(out=out_d[:, 2:4, :], in_=ot[:, 2:4, :])

            ps3, a3, w3 = qk_seg(3)
            evac_seg(ps3, a3, w3, EV, EV)
            v_pair(6)

            ee5 = sem_block(5, EV)
            av4b = avps.tile([128, 4, 65], f32, tag="av4")
            av_block(4, ee4, av4b, 0)

            v_pair(8)
            ee6 = sem_block(6, EV)
            av_block(5, ee5, av4b, 1)
            close_pair(4, av4b, 0, EV)
            nc.sync.dma_start(out=out_d[:, 4:6, :], in_=ot[:, 4:6, :])

            ee7 = sem_block(7, EV)
            av_block(6, ee6, av4b, 2)
            av_block(7, ee7, av4b, 3)
            close_pair(6, av4b, 2, EV)
            nc.sync.dma_start(out=out_d[:, 6:8, :], in_=ot[:, 6:8, :])

    nc.compile()
    return nc


def _get_nc():
    if "nc" not in _CACHE:
        _ensure_hooks()
        _CACHE["nc"] = _build_nc()
    return _CACHE["nc"]


def _host_inputs(inputs, wq, wk, wv):
    bf16 = ml_dtypes.bfloat16
    x = np.asarray(inputs, dtype=np.float32)

    # wqkv[p, dc, 0:64]=wq, [64:128]=wk, [128:192]=wv  (rows dc*128+p)
    wcat = np.concatenate(
        [np.asarray(wq), np.asarray(wk), np.asarray(wv)], axis=1
    ).astype(np.float32)                                     # [512, 192]
    wqkv = np.ascontiguousarray(
        wcat.reshape(NDC, 128, 192).transpose(1, 0, 2)
    )                                                        # [128, 4, 192]

    in_maps = []
    for i in range(8):
        b, c = divmod(i, 4)
        s0 = c * SS
        xp = np.zeros((NP, D), np.float32)
        lo = max(0, s0 - HP)
        hi = min(S, s0 + SS + HP)
        xp[lo - (s0 - HP) : hi - (s0 - HP)] = x[b, lo:hi]
        x4 = xp.T.reshape(NDC, 128, NP).transpose(1, 0, 2)  # [128, 4, 1280]
        t0 = np.concatenate([wqkv, x4[:, :, 0:128]], axis=2)  # [128,4,320]
        m = {
            "t0": np.ascontiguousarray(t0).astype(bf16),
            "s0": np.ascontiguousarray(x4[:, :, 128:512]).astype(bf16),
            "t1": np.ascontiguousarray(x4[:, :, 512:896]).astype(bf16),
            "s1": np.ascontiguousarray(x4[:, :, 896:1280]).astype(bf16),
        }
        in_maps.append(m)
    return in_maps


def run_sharded(inputs, wq, wk, wv, trace=False, trace_cores=None):
    """Run the SPMD kernel; returns (out [B,S,HD] f32, BassKernelResults)."""
    _ensure_hooks()
    import concourse.bass_utils as bass_utils

    nc = _get_nc()
    in_maps = _host_inputs(inputs, wq, wk, wv)
    res = bass_utils.run_bass_kernel_spmd(
        nc,
        in_maps,
        core_ids=list(range(8)),
        trace=trace,
        trace_cores=trace_cores,
    )
    out = np.empty((B, S, HD), np.float32)
    for i in range(8):
        b, c = divmod(i, 4)
        o = res.results[i]["outp"]                           # [128, 8, 65]
        o = o[:, :, 0:HD] / o[:, :, HD : HD + 1]
        out[b, c * SS : (c + 1) * SS] = o.transpose(1, 0, 2).reshape(SS, HD)
    return out, res


def kernel(inputs, wq, wk, wv):
    out, _ = run_sharded(inputs, wq, wk, wv, trace=False)
    return out


# revision 11
# speedup vs baseline: 1.0646x; 1.0312x over previous
"""Sparse (sliding-window) attention head on 8 TRN2 NeuronCores.

Reference computation (B=2, S=4096, D=512, HD=64, SCALE=128):
    q = x @ wq ; k = x @ wk ; v = x @ wv          [B,S,64]
    scores[b,s,w] = q[b,s] . k[b,s-128+w] / 8     w in [0,256), zero-padded OOB
    out = softmax_w(scores) @ v_window            [B,S,64]

Sharding: 8 shards = (batch b, 1024-seq chunk c). Each shard gets a
zero-padded 128-halo of x on both sides, which reproduces the reference's
zero-padded (not masked) window semantics exactly. All compute is local.

v10 layout (per core):
    Inputs arrive bf16 as 4 fat tensors (>=2.5KB/partition descriptors for
    ~200GB/s/queue): T0=[wqk|wv|x 0:128] + T1=[x 512:896] on the SP queue,
    S0=[x 128:512] + S1=[x 896:1280] on the ACT queue.  Warmup MMs hold the
    PE HAM window busy from ~7.1us until T0 lands (~9.6us) so the clock
    reaches 2.4GHz early.  qk proj per segment (wq|wk packed lhsT), k/q
    evacuated with partition-shifting copies (ACT early, DVE late).  Band
    masks from GPSIMD iota + DVE compares.  All sc->exp->mask chains are
    emitted as early as data allows so the trailing AV accumulations run
    back-to-back with no latency serialization; per-pair norms (batched
    reciprocal + scale) stagger output DMAs, ending with one paired {6,7}
    close + DMA.
"""

import sys
import types

import numpy as np
import ml_dtypes

B, S, D = 2, 4096, 512
HD = 64
SCALE = 128
SS = S // 4          # 1024 positions per shard
HP = SCALE           # halo padding each side
NP = SS + 2 * HP     # 1280 padded positions
NKC = NP // 128      # 10 key chunks
NQB = SS // 128      # 8 query blocks
NDC = D // 128       # 4 d-chunks

# x col segments (start, width); seg0 rides inside T0 after the weights
XSEGS = [(0, 128), (128, 384), (512, 384), (896, 384)]
N_WARMUP = 12

_CACHE = {}


def _ensure_hooks():
    """Register the axon NTFF profile hook; keep artifacts local."""
    if "antenv.axon_hooks" not in sys.modules:
        try:
            from trn_agent_boot.trn_boot import _ntff_profile_via_ctypes

            m = types.ModuleType("antenv.axon_hooks")
            m.get_axon_ntff_profile_hook = lambda: _ntff_profile_via_ctypes(
                "/opt/axon/libaxon_pjrt.so"
            )
            sys.modules["antenv.axon_hooks"] = m
        except Exception:
            pass
    import concourse.bass_utils as bass_utils

    bass_utils.upload_artifacts = lambda tmpdir: tmpdir


def _build_nc():
    import concourse.mybir as mybir
    import concourse.tile as tile
    from concourse import bacc

    bf = mybir.dt.bfloat16
    f32 = mybir.dt.float32
    AF = mybir.ActivationFunctionType

    nc = bacc.Bacc("TRN2", target_bir_lowering=False, debug=False, num_devices=8)

    t0_d = nc.dram_tensor("t0", [128, NDC, 320], bf, kind="ExternalInput")
    s0_d = nc.dram_tensor("s0", [128, NDC, 384], bf, kind="ExternalInput")
    t1_d = nc.dram_tensor("t1", [128, NDC, 384], bf, kind="ExternalInput")
    s1_d = nc.dram_tensor("s1", [128, NDC, 384], bf, kind="ExternalInput")
    out_d = nc.dram_tensor("outp", [128, NQB, HD + 1], f32, kind="ExternalOutput")

    with tile.TileContext(nc) as tc:
        with (
            tc.tile_pool(name="consts", bufs=1) as consts,
            tc.tile_pool(name="xtp", bufs=1) as xtp,
            tc.tile_pool(name="qkp", bufs=1) as qkp,
            tc.tile_pool(name="vgp", bufs=1) as vgp,
            tc.tile_pool(name="exp_p", bufs=6) as exp_p,
            tc.tile_pool(name="fin", bufs=4) as fin,
            tc.tile_pool(name="qkps", bufs=2, space="PSUM") as qkps,
            tc.tile_pool(name="vps", bufs=1, space="PSUM") as vps,
            tc.tile_pool(name="scps", bufs=3, space="PSUM") as scps,
            tc.tile_pool(name="avps", bufs=2, space="PSUM") as avps,
        ):
            # ---- DMAs first (fat descriptors, consumption order) ----
            t0 = xtp.tile([128, NDC, 320], bf, tag="t0")
            s0 = xtp.tile([128, NDC, 384], bf, tag="s0")
            t1 = xtp.tile([128, NDC, 384], bf, tag="t1")
            s1 = xtp.tile([128, NDC, 384], bf, tag="s1")

            nc.sync.dma_start(out=t0, in_=t0_d[:, :, :])
            nc.scalar.dma_start(out=s0, in_=s0_d[:, :, :])
            nc.sync.dma_start(out=t1, in_=t1_d[:, :, :])
            nc.scalar.dma_start(out=s1, in_=s1_d[:, :, :])

            def wqk(dc):
                return t0[:, dc, 0:128]

            def wv_(dc):
                return t0[:, dc, 128:192]

            # x segment view: seg si, dc, col slice (cols local to seg)
            segt = [
                (t0, 192),   # seg0: x cols 0:128 at offset 192
                (s0, 0),
                (t1, 0),
                (s1, 0),
            ]

            def xs(si, dc, a, b):
                t, off = segt[si]
                return t[:, dc, off + a : off + b]

            # ---- consts: warmup garbage (DVE), exp-table trigger (ACT),
            # band mask via GPSIMD iota + DVE compares, vaug ones (GPSIMD) ----
            garb = consts.tile([128, 260], bf, tag="garb")
            nc.vector.memset(garb, 0.5)
            zz = consts.tile([128, 1], f32, tag="zz")
            nc.vector.memset(zz, 0.0)
            ez = consts.tile([128, 1], f32, tag="ez")
            nc.scalar.activation(ez, zz, AF.Exp)

            vaug = vgp.tile([128, NKC, 66], bf, tag="vaug")
            nc.gpsimd.memset(vaug[:, :, 64:66], 1.0)
            mi = consts.tile([128, 128], mybir.dt.int16, tag="mi")
            nc.gpsimd.iota(mi, pattern=[[1, 128]], base=0, channel_multiplier=-1)
            amask = consts.tile([128, 2, 128], f32, tag="amask")
            nc.vector.tensor_scalar(
                amask[:, 0, :], mi, 0, None, mybir.AluOpType.is_gt
            )
            nc.vector.tensor_scalar(
                amask[:, 1, :], mi, 0, None, mybir.AluOpType.is_le
            )
            nc.vector.tensor_scalar(
                amask, amask, -1.0e5, None, mybir.AluOpType.mult
            )

            qT_s = qkp.tile([64, SS], bf, tag="qT")
            kT_s = qkp.tile([64, NP], bf, tag="kT")
            ot = fin.tile([128, NQB, HD + 1], f32, tag="ot")

            # ---- PE warmup: hold HAM busy until real data lands ----
            for i in range(N_WARMUP):
                wp = qkps.tile([128, 384], f32, tag="qkps")
                nc.tensor.matmul(
                    wp[:, 0:256],
                    lhsT=garb[:, 0:128],
                    rhs=garb[:, 0:256],
                    start=True,
                    stop=True,
                )

            # ---- helpers ----
            def qk_seg(si):
                a, w = XSEGS[si]
                ps = qkps.tile([128, 384], f32, tag="qkps")
                for dc in range(NDC):
                    nc.tensor.matmul(
                        ps[:, :w],
                        lhsT=wqk(dc),
                        rhs=xs(si, dc, 0, w),
                        start=(dc == 0),
                        stop=(dc == NDC - 1),
                    )
                return ps, a, w

            def evac_seg(ps, a, w, keng, qeng):
                kcp = keng.copy if keng is nc.scalar else keng.tensor_copy
                qcp = qeng.copy if qeng is nc.scalar else qeng.tensor_copy
                kcp(kT_s[:, a : a + w], ps[64:128, :w])
                qa, qb_ = max(a, HP), min(a + w, HP + SS)
                if qa < qb_:
                    qcp(qT_s[:, qa - HP : qb_ - HP], ps[0:64, qa - a : qb_ - a])

            def v_chunk(kc, vp, j):
                c0 = kc * 128
                for si, (a, w) in enumerate(XSEGS):
                    if a <= c0 < a + w:
                        break
                off = c0 - a
                for dc in range(NDC):
                    nc.tensor.matmul(
                        vp[:, j, :],
                        lhsT=xs(si, dc, off, off + 128),
                        rhs=wv_(dc),
                        start=(dc == 0),
                        stop=(dc == NDC - 1),
                    )

            def v_pair(c):
                vp = vps.tile([128, 2, HD], f32, tag="vp")
                v_chunk(c, vp, 0)
                v_chunk(c + 1, vp, 1)
                nc.vector.tensor_copy(vaug[:, c : c + 2, 0:64], vp)

            def sem_block(qb, meng=None):
                """sc -> additive band-mask (pre-exp) -> exp for one block."""
                sc = scps.tile([128, 3, 128], f32, tag="sc")
                for c in range(3):
                    nc.tensor.matmul(
                        sc[:, c, :],
                        lhsT=kT_s[:, (qb + c) * 128 : (qb + c + 1) * 128],
                        rhs=qT_s[:, qb * 128 : (qb + 1) * 128],
                        start=True,
                        stop=True,
                    )
                nc.vector.tensor_add(sc[:, 0:3:2, :], sc[:, 0:3:2, :], amask)
                ex = exp_p.tile([128, 3, 128], bf, tag="ex")
                nc.scalar.activation(ex[:, :, :], sc, AF.Exp, scale=0.125)
                return ex

            def av_block(qb, ex, av4, j):
                nc.tensor.matmul(
                    av4[:, j, :],
                    lhsT=ex[:, 1, :],
                    rhs=vaug[:, qb + 1, 0:65],
                    start=True,
                    stop=False,
                )
                nc.tensor.matmul(
                    av4[:, j, :],
                    lhsT=ex[:, 0, :],
                    rhs=vaug[:, qb, 0:65],
                    start=False,
                    stop=False,
                )
                nc.tensor.matmul(
                    av4[:, j, :],
                    lhsT=ex[:, 2, :],
                    rhs=vaug[:, qb + 2, 0:65],
                    start=False,
                    stop=True,
                )

            def close_pair(b, av4, j, eng):
                """Evacuate numerator+denominator; the host does the divide."""
                cp = eng.copy if eng is nc.scalar else eng.tensor_copy
                cp(ot[:, b : b + 2, :], av4[:, j : j + 2, :])

            def filler():
                wp = qkps.tile([128, 384], f32, tag="qkps")
                nc.tensor.matmul(
                    wp[:, 0:256],
                    lhsT=garb[:, 0:128],
                    rhs=garb[:, 0:256],
                    start=True,
                    stop=True,
                )

            # ---- pipeline ----
            EA, EV, EG = nc.scalar, nc.vector, nc.gpsimd

            ps0, a0, w0 = qk_seg(0)
            evac_seg(ps0, a0, w0, EA, EA)
            vp01 = vps.tile([128, 2, HD], f32, tag="vp")
            v_chunk(0, vp01, 0)
            filler()
            filler()
            filler()
            v_chunk(1, vp01, 1)
            nc.vector.tensor_copy(vaug[:, 0:2, 0:64], vp01)

            ps1, a1, w1 = qk_seg(1)
            evac_seg(ps1, a1, w1, EV, EV)
            v_pair(2)

            ee0 = sem_block(0)
            ps2, a2, w2 = qk_seg(2)
            evac_seg(ps2, a2, w2, EA, EA)
            v_pair(4)

            ee1 = sem_block(1)
            av4a = avps.tile([128, 4, 65], f32, tag="av4")
            av_block(0, ee0, av4a, 0)

            ee2 = sem_block(2)
            av_block(1, ee1, av4a, 1)
            close_pair(0, av4a, 0, EV)
            nc.sync.dma_start(out=out_d[:, 0:2, :], in_=ot[:, 0:2, :])

            ee3 = sem_block(3)
            av_block(2, ee2, av4a, 2)

            ee4 = sem_block(4)
            av_block(3, ee3, av4a, 3)
            close_pair(2, av4a, 2, EA)
            nc.scalar.dma_start(out=out_d[:, 2:4, :], in_=ot[:, 2:4, :])

            ps3, a3, w3 = qk_seg(3)
            evac_seg(ps3, a3, w3, EV, EV)
            v_pair(6)

            ee5 = sem_block(5)
            av4b = avps.tile([128, 4, 65], f32, tag="av4")
            av_block(4, ee4, av4b, 0)

            v_pair(8)
            ee6 = sem_block(6)
            av_block(5, ee5, av4b, 1)
            close_pair(4, av4b, 0, EV)
            nc.sync.dma_start(out=out_d[:, 4:6, :], in_=ot[:, 4:6, :])

            ee7 = sem_block(7)
            av_block(6, ee6, av4b, 2)
            av_block(7, ee7, av4b, 3)
            close_pair(6, av4b, 2, EV)
            nc.scalar.dma_start(out=out_d[:, 6:8, :], in_=ot[:, 6:8, :])

    nc.compile()
    return nc


def _get_nc():
    if "nc" not in _CACHE:
        _ensure_hooks()
        _CACHE["nc"] = _build_nc()
    return _CACHE["nc"]


def _host_inputs(inputs, wq, wk, wv):
    bf16 = ml_dtypes.bfloat16
    x = np.asarray(inputs, dtype=np.float32)

    # wqkv[p, dc, 0:64]=wq, [64:128]=wk, [128:192]=wv  (rows dc*128+p)
    wcat = np.concatenate(
        [np.asarray(wq), np.asarray(wk), np.asarray(wv)], axis=1
    ).astype(np.float32)                                     # [512, 192]
    wqkv = np.ascontiguousarray(
        wcat.reshape(NDC, 128, 192).transpose(1, 0, 2)
    )                                                        # [128, 4, 192]

    in_maps = []
    for i in range(8):
        b, c = divmod(i, 4)
        s0 = c * SS
        xp = np.zeros((NP, D), np.float32)
        lo = max(0, s0 - HP)
        hi = min(S, s0 + SS + HP)
        xp[lo - (s0 - HP) : hi - (s0 - HP)] = x[b, lo:hi]
        x4 = xp.T.reshape(NDC, 128, NP).transpose(1, 0, 2)  # [128, 4, 1280]
        t0 = np.concatenate([wqkv, x4[:, :, 0:128]], axis=2)  # [128,4,320]
        m = {
            "t0": np.ascontiguousarray(t0).astype(bf16),
            "s0": np.ascontiguousarray(x4[:, :, 128:512]).astype(bf16),
            "t1": np.ascontiguousarray(x4[:, :, 512:896]).astype(bf16),
            "s1": np.ascontiguousarray(x4[:, :, 896:1280]).astype(bf16),
        }
        in_maps.append(m)
    return in_maps


def run_sharded(inputs, wq, wk, wv, trace=False, trace_cores=None):
    """Run the SPMD kernel; returns (out [B,S,HD] f32, BassKernelResults)."""
    _ensure_hooks()
    import concourse.bass_utils as bass_utils

    nc = _get_nc()
    in_maps = _host_inputs(inputs, wq, wk, wv)
    res = bass_utils.run_bass_kernel_spmd(
        nc,
        in_maps,
        core_ids=list(range(8)),
        trace=trace,
        trace_cores=trace_cores,
    )
    out = np.empty((B, S, HD), np.float32)
    for i in range(8):
        b, c = divmod(i, 4)
        o = res.results[i]["outp"]                           # [128, 8, 65]
        o = o[:, :, 0:HD] / o[:, :, HD : HD + 1]
        out[b, c * SS : (c + 1) * SS] = o.transpose(1, 0, 2).reshape(SS, HD)
    return out, res


def kernel(inputs, wq, wk, wv):
    out, _ = run_sharded(inputs, wq, wk, wv, trace=False)
    return out
